# revision 2
# baseline (speedup 1.0000x reference)
"""Trainium2 Bass kernel (v4 pipeline) for nn_ExpNegL2 (exp(-||a_n - t_n||) retrieval).

Full inputs: audio [32, 4096, 512] f32, text [32, 64, 512] f32.
Output: [32, 64, 4096] f32 = exp(-sqrt(2 - 2 * <normalize(text), normalize(audio)>)).
Sharding: data-parallel over batch, 4 batches per core across 8 cores.

v4 pipeline (per core):
  SWDGE cast-loads audio bf16 in (n p) row order -> PE transposes the RAW
  tiles (no normalize pass!) -> one wide PSUM->SBUF copy-back per t-block ->
  PE matmul vs normalized text gives RAW dots. The audio 1/||a|| folds into
  the post stage instead: fused square+accum ops (1 per sub-tile, split
  ACT/DVE/Pool) -> per-pair rsqrt -> PE-transpose + SWDGE cast-flatten turns
  the per-partition inv into a [1, t] bf16 row -> K=1 ones-matmul broadcasts
  it into PSUM -> one DVE scalar_tensor_tensor per pair computes
  y = -2*dots*inv -> ACT chain Ln(y+2), Exp(0.5), Exp(-1) (all table 6,
  preloaded once: zero activation-table reloads).
"""

import os
import sys

sys.path.insert(0, "/opt/trn_rl_repo")

import contextlib

import numpy as np

import concourse.bacc as bacc
import concourse.tile as tile
from concourse import mybir
from concourse import bass_utils
from concourse.masks import make_identity


def _env(name, default):
    # graded artifact: knobs are fixed at their tuned values
    return int(os.environ.get(name, default)) if os.environ.get("KTUNE") else default


N_CORES = 8
B, T, M, D = 32, 4096, 64, 512
B_LOC = B // N_CORES
TB = 512
NT = T // TB
NSUB = TB // 128
NCH = D // 128

# squares: sub-tile n handled by ACT (n < SQ_ACT), Pool (next SQ_POOL), DVE rest
SQ_ACT = _env("KSQ_ACT", 1)
SQ_POOL = _env("KSQ_POOL", 0)
# copy-back engine pattern by t-block index: 'D'=DVE, 'A'=ACT
CB_PAT = os.environ.get("KCB", "DDDA")
TX_PRE = _env("KTXPRE", 0)   # prefetch next batch's text mid-batch
TX_SW = _env("KTXSW", 0)     # text load via SWDGE instead of HWDGE
OST1 = _env("KOST1", 0)      # per-batch 1MB stores instead of 2-batch 2MB

F32 = mybir.dt.float32
BF16 = mybir.dt.bfloat16
MUL = mybir.AluOpType.mult


def _body(ctx, tc, out, audio, text, repeat=1):
    nc = tc.nc
    Ln = mybir.ActivationFunctionType.Ln
    Exp = mybir.ActivationFunctionType.Exp
    Square = mybir.ActivationFunctionType.Square

    from concourse.hw_specs import get_activation_tables
    tbl = list(get_activation_tables(nc.m.arch)).index("natural_log_exp_and_others")
    nc.scalar.add_instruction(mybir.InstLoadActFuncSet(
        name="atl_preload", ins=[], outs=[], act_func_set_id=tbl))

    singles = ctx.enter_context(tc.tile_pool(name="singles", bufs=1))
    ident = singles.tile([128, 128], BF16)
    make_identity(nc, ident)
    two = singles.tile([128, 1], F32)
    nc.vector.memset(two, 2.0)
    ones = singles.tile([1, M], BF16)
    nc.vector.memset(ones, 1.0)

    tx_pool = ctx.enter_context(tc.tile_pool(name="tx", bufs=_env("KB_TX", 2)))
    nat_pool = ctx.enter_context(tc.tile_pool(name="nat", bufs=_env("KB_NAT", 4)))
    at_pool = ctx.enter_context(tc.tile_pool(name="at", bufs=_env("KB_AT", 3)))
    scr_pool = ctx.enter_context(tc.tile_pool(name="scr", bufs=_env("KB_SCR", 6)))
    small_pool = ctx.enter_context(
        tc.tile_pool(name="small", bufs=_env("KB_SMALL", 2)))
    invrow_pool = ctx.enter_context(
        tc.tile_pool(name="invrow", bufs=_env("KB_INVR", 2)))
    post_pool = ctx.enter_context(
        tc.tile_pool(name="post", bufs=_env("KB_POST", 8)))
    ostage_pool = ctx.enter_context(
        tc.tile_pool(name="ostage", bufs=_env("KB_OST", 2)))
    psum_mm = ctx.enter_context(
        tc.tile_pool(name="psum_mm", bufs=_env("KB_PSMM", 2), space="PSUM"))
    psum_tr = ctx.enter_context(
        tc.tile_pool(name="psum_tr", bufs=_env("KB_PSTR", 2), space="PSUM"))
    psum_bc = ctx.enter_context(
        tc.tile_pool(name="psum_bc", bufs=_env("KB_PSBC", 1), space="PSUM"))
    psum_tx = ctx.enter_context(
        tc.tile_pool(name="psum_tx", bufs=1, space="PSUM"))

    def prep_text(b):
        # ---- text: load, l2-normalize, cast bf16, PE-transpose to [d, c, m]
        txf = tx_pool.tile([M, D], F32)
        if TX_SW:
            nc.gpsimd.dma_start(out=txf, in_=text[b])
        else:
            nc.sync.dma_start(out=txf, in_=text[b])
        t_scr = tx_pool.tile([M, D], BF16)
        t_ssq = tx_pool.tile([M, 1], F32)
        nc.scalar.activation(t_scr, txf, Square, accum_out=t_ssq)
        t_ln = tx_pool.tile([M, 1], F32)
        nc.scalar.activation(t_ln, t_ssq, Ln)
        t_inv = tx_pool.tile([M, 1], F32)
        nc.scalar.activation(t_inv, t_ln, Exp, scale=-0.5)
        txn = tx_pool.tile([M, D], BF16)
        nc.vector.tensor_scalar_mul(txn, txf, t_inv)
        pt_t = psum_tx.tile([128, NCH, M], BF16)
        for c in range(NCH):
            nc.tensor.transpose(
                pt_t[:, c, :], txn[:, c * 128:(c + 1) * 128], ident[0:M, 0:M])
        tnt = tx_pool.tile([128, NCH, M], BF16)
        nc.vector.tensor_copy(tnt, pt_t)
        return tnt

    ostage = None
    blist = [b for _ in range(repeat) for b in range(B_LOC)]
    tnt = prep_text(blist[0]) if TX_PRE else None
    next_tnt = None
    for bi, b in enumerate(blist):
        if TX_PRE:
            if bi > 0:
                tnt = next_tnt
        else:
            tnt = prep_text(b)

        if OST1:
            ostage = ostage_pool.tile([M, T], F32, name="ostage1")
            po = 0
        else:
            if b % 2 == 0:
                ostage = ostage_pool.tile([128, T], F32)
            po = (b % 2) * M

        ssq_b = small_pool.tile([128, NT, NSUB], F32)
        inv_b = small_pool.tile([128, NT, NSUB], BF16)
        inv_row = invrow_pool.tile([1, T], BF16)

        ats = {}
        dots = {}
        for i in range(NT + 1):
            if i >= 1:
                # ---- matmuls for t-block i-1 (raw audio vs normalized text)
                tbm = i - 1
                if tbm % 2 == 0:
                    dots[tbm // 2] = psum_mm.tile([2 * M, TB], F32, name="dots")
                d2 = dots[tbm // 2]
                half = (tbm % 2) * M
                atm = ats.pop(tbm)
                for c in range(NCH):
                    nc.tensor.matmul(
                        d2[half:half + M, :], tnt[:, c, :], atm[:, c, :],
                        start=(c == 0), stop=(c == NCH - 1),
                        tile_position=(0, half),
                    )

            if TX_PRE and i == 2 and bi + 1 < len(blist):
                next_tnt = prep_text(blist[bi + 1])

            if i < NT:
                # ---- load raw bf16 audio; (n p): partition p sub n = row n*128+p
                src = audio[b, i * TB:(i + 1) * TB, :].rearrange(
                    "(n p) d -> p n d", p=128)
                nat = nat_pool.tile([128, NSUB, D], BF16)
                nc.gpsimd.dma_start(out=nat, in_=src)

                # ---- fused square+accum per sub-tile (engine split)
                for n in range(NSUB):
                    acc = ssq_b[:, i, n:n + 1]
                    if n < SQ_ACT:
                        scr = scr_pool.tile([128, D], BF16)
                        nc.scalar.activation(
                            scr, nat[:, n, :], Square, accum_out=acc)
                    elif n < SQ_ACT + SQ_POOL:
                        scr = scr_pool.tile([128, D], BF16)
                        nc.gpsimd.scalar_tensor_tensor(
                            scr, nat[:, n, :], 1.0, nat[:, n, :],
                            op0=MUL, op1=MUL, accum_out=acc)
                    else:
                        scr = scr_pool.tile([128, D], BF16)
                        nc.vector.scalar_tensor_tensor(
                            scr, nat[:, n, :], 1.0, nat[:, n, :],
                            op0=MUL, op1=MUL, accum_out=acc)

                # ---- PE transposes of RAW audio into one wide PSUM tile
                pt = psum_tr.tile([128, NCH, TB], BF16)
                for n in range(NSUB):
                    for c in range(NCH):
                        nc.tensor.transpose(
                            pt[:, c, n * 128:(n + 1) * 128],
                            nat[:, n, c * 128:(c + 1) * 128], ident)
                at = at_pool.tile([128, NCH, TB], BF16)
                if CB_PAT[i % len(CB_PAT)] == "A":
                    nc.scalar.copy(at, pt)
                else:
                    nc.vector.tensor_copy(at, pt)
                ats[i] = at

                if i % 2 == 1:
                    # ---- pair p=(i-1)//2: rsqrt, inv transpose, cast-flatten
                    p = i // 2
                    sl = slice(2 * p, 2 * p + 2)
                    rs = small_pool.tile([128, 2 * NSUB], F32, name="rs")
                    nc.scalar.activation(
                        rs, ssq_b[:, sl, :].rearrange("p a b -> p (a b)"), Ln)
                    ivs = inv_b[:, sl, :].rearrange("p a b -> p (a b)")
                    nc.scalar.activation(ivs, rs, Exp, scale=-0.5)
                    # PE transpose [128, 8] bf16 -> [8, 128] into the shared
                    # psum_tx tile
                    ptx = psum_tx.tile([128, NCH, M], BF16, name="pt_t")
                    vw = ptx.rearrange("p a b -> p (a b)")
                    nc.tensor.transpose(vw[0:2 * NSUB, 0:128], ivs, ident)
                    ivT = scr_pool.tile([2 * NSUB, 128], BF16, name="ivT")
                    nc.vector.tensor_copy(ivT, vw[0:2 * NSUB, 0:128])
                    # SWDGE flatten (i,n,p) -> t order (plain bf16 copy)
                    nc.gpsimd.dma_start(
                        out=inv_row[0:1, 2 * p * TB:(2 * p + 2) * TB], in_=ivT)

            if i >= 2 and i % 2 == 0:
                # ---- post for pair p=(i-2)//2: scale by inv via K=1 bcast
                p = (i - 2) // 2
                d2p = dots.pop(p)
                invb2 = psum_bc.tile([2 * M, TB], F32, name="invb2")
                for h in range(2):
                    tbh = 2 * p + h
                    nc.tensor.matmul(
                        invb2[h * M:(h + 1) * M, :], ones,
                        inv_row[0:1, tbh * TB:(tbh + 1) * TB],
                        start=True, stop=True, tile_position=(0, h * M))
                invb_s = post_pool.tile([2 * M, TB], F32)
                nc.vector.tensor_copy(invb_s, invb2)
                y = post_pool.tile([2 * M, TB], F32)
                nc.vector.scalar_tensor_tensor(
                    y, d2p, -2.0, invb_s, op0=MUL, op1=MUL)
                lnz = post_pool.tile([2 * M, TB], F32)
                nc.scalar.activation(lnz, y, Ln, bias=two)
                dist = post_pool.tile([2 * M, TB], F32)
                nc.scalar.activation(dist, lnz, Exp, scale=0.5)
                for h in range(2):
                    tbh = 2 * p + h
                    nc.scalar.activation(
                        ostage[po:po + M, tbh * TB:(tbh + 1) * TB],
                        dist[h * M:(h + 1) * M], Exp, scale=-1.0)

        if OST1:
            nc.sync.dma_start(out=out[b], in_=ostage)
        elif b % 2 == 1:
            dst = out[b - 1:b + 1].rearrange("b m t -> (b m) t")
            nc.sync.dma_start(out=dst, in_=ostage)


_NC_CACHE = {}


def _build(repeat=1):
    if repeat in _NC_CACHE:
        return _NC_CACHE[repeat]
    nc = bacc.Bacc(
        "TRN2", target_bir_lowering=False, debug=False,
        enable_asserts=False, num_devices=N_CORES,
    )
    audio = nc.dram_tensor("audio", [B_LOC, T, D], F32, kind="ExternalInput").ap()
    text = nc.dram_tensor("text", [B_LOC, M, D], F32, kind="ExternalInput").ap()
    out = nc.dram_tensor("out", [B_LOC, M, T], F32, kind="ExternalOutput").ap()
    with tile.TileContext(nc) as tc:
        with contextlib.ExitStack() as ctx:
            _body(ctx, tc, out, audio, text, repeat=repeat)
    nc.compile()
    _NC_CACHE[repeat] = nc
    return nc


def kernel(audio: np.ndarray, text: np.ndarray) -> np.ndarray:
    nc = _build()
    in_maps = []
    for i in range(N_CORES):
        sl = slice(i * B_LOC, (i + 1) * B_LOC)
        in_maps.append({
            "audio": np.ascontiguousarray(audio[sl], dtype=np.float32),
            "text": np.ascontiguousarray(text[sl], dtype=np.float32),
        })
    res = bass_utils.run_bass_kernel_spmd(nc, in_maps, core_ids=list(range(N_CORES)))
    return np.concatenate([r["out"] for r in res.results], axis=0)
